# revision 3
# baseline (speedup 1.0000x reference)
"""Trainium2 Bass kernel: CentroidModule (VQ codebook update).

Strategy (data-parallel over B across 8 NeuronCores, 8192 tokens/core):
  - Host prep (layout/dtype only): batch pre-transposed to d-major fp32r
    (12-bit-rounded fp32; runs on the PE at bf16 speed) for the scores
    lhsT, plus a token-major fp16 copy; protos normalized + transposed
    once (K*D work).
  - Scores on raw (unnormalized) tokens: argmin_k ||b/m - p_k||^2 ==
    argmax_k (S_tk - m_t*q_k) with S = batch @ p^T, q = 0.5||p||^2,
    m = max(1, len).  Per 128-token tile the PE does 3 fp32r matmuls
    into one PSUM tile [128, 512]: two 128-d blocks of S plus a C=6
    bias matmul whose lhsT holds transposed (m-16) columns and 16.0
    consts, rhs holds -q split hi/lo (12-bit pieces) -- bias exact to
    ~1e-6 while every PE pass runs at 1 cycle/row.
  - mx = row max (DVE reduce from PSUM); A = Sign(mx - u) on ACT =
    {0 at argmax, 1 else} (inverted one-hot, resolved on host);
    bn = fp16(b16 * s) via ACT Copy with per-partition scale.
  - Per-token stats: ss via one DVE scalar_tensor_tensor (b*1)*b with
    sum-accum; sqrt/recip/clip batched 4 tiles at a time to amortize
    engine fixed overheads.
  - segment sums: acc[K,257] += A^T @ [bn | 1] on PE, PSUM-accumulated
    over all 64 tiles (4 K-blocks, fp16).
  - Host: raw -> true sums via total/511 - raw (each token appears in
    exactly 511 of the inverted rows), add running stats, normalize.
"""

import numpy as np
from contextlib import ExitStack

import concourse.bacc as bacc
import concourse.mybir as mybir
import concourse.tile as tile
from concourse.bass_utils import run_bass_kernel_spmd

B, T, D, K = 64, 1024, 256, 512
NCORES = 8
TPC = (B * T) // NCORES      # tokens per core = 8192
NT = TPC // 128              # 64 token tiles per core
F32 = mybir.dt.float32
F32R = mybir.dt.float32r
FP16 = mybir.dt.float16
AF = mybir.ActivationFunctionType
OP = mybir.AluOpType


def _body(tc, part_d, bT_d, btk_d, pT_d, q6_d, id_d, c16_d):
    nc = tc.nc
    with ExitStack() as ctx:
        const = ctx.enter_context(tc.tile_pool(name="const", bufs=1))
        work = ctx.enter_context(tc.tile_pool(name="work", bufs=4))
        small = ctx.enter_context(tc.tile_pool(name="small", bufs=6))
        ppt = ctx.enter_context(tc.tile_pool(name="ppt", bufs=3, space="PSUM"))
        ppb = ctx.enter_context(tc.tile_pool(name="ppb", bufs=1, space="PSUM"))
        psums = ctx.enter_context(tc.tile_pool(name="psums", bufs=1, space="PSUM"))

        # ---------------- constants ----------------
        pT = [const.tile([128, K], F32R, tag=f"pT{h}", name=f"pT{h}")
              for h in (0, 1)]
        for h in (0, 1):
            nc.sync.dma_start(pT[h][:], pT_d[h * 128:(h + 1) * 128, :])
        q6 = [const.tile([6, K], F32R, tag=f"q6_{j}", name=f"q6_{j}")
              for j in range(4)]
        for j in range(4):
            nc.sync.dma_start(q6[j][:], q6_d[6 * j:6 * j + 6, :])
        ident = const.tile([128, 128], F32R, tag="ident", name="ident")
        nc.sync.dma_start(ident[:], id_d[:, :])

        # ---------------- segment-sum accumulators ----------------
        acc = [
            psums.tile([128, D + 1], F32, tag=f"acc{kt}", name=f"acc{kt}")
            for kt in range(4)
        ]

        st = {}
        grp = {}

        def stage_load(it):
            v = st.setdefault(it, {})
            bt0 = work.tile([128, 128], F32R, tag="bt0", bufs=8, name=f"bt0_{it}")
            bt1 = work.tile([128, 128], F32R, tag="bt1", bufs=8, name=f"bt1_{it}")
            nc.sync.dma_start(bt0[:], bT_d[0:128, it * 128:(it + 1) * 128])
            nc.sync.dma_start(bt1[:], bT_d[128:256, it * 128:(it + 1) * 128])
            b16 = work.tile([128, D], FP16, tag="b16", bufs=10, name=f"b16_{it}")
            nc.sync.dma_start(b16[:], btk_d[it * 128:(it + 1) * 128, :])
            v["bt0"], v["bt1"], v["b16"] = bt0, bt1, b16

        def stage_stat(it):
            g, j = it >> 2, it & 3
            if j == 0:
                gv = grp.setdefault(g, {})
                gv["ss4"] = small.tile([128, 4], F32, tag="ss4", bufs=3,
                                       name=f"ss4_{g}")
                gv["tcol"] = work.tile([128, 6], F32R, tag="tcol", bufs=2,
                                       name=f"tcol_{g}")
                nc.sync.dma_start(gv["tcol"][:, 4:6], c16_d[:, :])
            gv = grp[g]
            sq = work.tile([128, D], FP16, tag="sq", bufs=2, name=f"sq{it}")
            nc.vector.scalar_tensor_tensor(
                sq[:], st[it]["b16"][:], 1.0, st[it]["b16"][:],
                op0=OP.mult, op1=OP.mult, accum_out=gv["ss4"][:, j:j + 1],
            )

        def stage_stat4(g):
            gv = grp[g]
            ssc4 = small.tile([128, 4], F32, tag="ssc4", bufs=3, name=f"ssc4_{g}")
            nc.gpsimd.tensor_scalar_max(ssc4[:], gv["ss4"][:], 1.0)
            m4 = small.tile([128, 4], F32, tag="m4", bufs=3, name=f"m4_{g}")
            nc.scalar.activation(m4[:], ssc4[:], AF.Sqrt)
            s4 = small.tile([128, 4], F32, tag="s4", bufs=3, name=f"s4_{g}")
            nc.vector.reciprocal(s4[:], m4[:])
            for j in range(4):
                nc.gpsimd.tensor_scalar_add(
                    gv["tcol"][:, j:j + 1], m4[:, j:j + 1], -16.0)
            gv["m4"], gv["s4"] = m4, s4

        def stage_stat4b(g):
            gv = grp[g]
            tct = ppb.tile([6, 128], F32R, tag="tct", name=f"tct_{g}")
            nc.tensor.transpose(tct[:], gv["tcol"][:], ident[:])
            tctr = work.tile([6, 128], F32R, tag="tctr", bufs=2, name=f"tctr_{g}")
            nc.vector.tensor_copy(tctr[:], tct[:])
            gv["tctr"] = tctr

        def stage_scores(it):
            v = st[it]
            g, j = it >> 2, it & 3
            tps = ppt.tile([128, K], F32, tag="tps", name=f"tps{it}")
            nc.tensor.matmul(tps[:], lhsT=v["bt0"][:], rhs=pT[0][:],
                             start=True, stop=False)
            nc.tensor.matmul(tps[:], lhsT=v["bt1"][:], rhs=pT[1][:],
                             start=False, stop=False)
            nc.tensor.matmul(tps[:], lhsT=grp[g]["tctr"][:], rhs=q6[j][:],
                             start=False, stop=True)
            v["tps"] = tps

        def stage_select(it):
            v = st[it]
            mx = small.tile([128, 1], F32, tag="mx", name=f"mx{it}")
            nc.vector.reduce_max(mx[:], v["tps"][:], axis=mybir.AxisListType.X)
            v["mx"] = mx

        def stage_onehot(it):
            v = st[it]
            g, j = it >> 2, it & 3
            A = work.tile([128, K], FP16, tag="A", bufs=3, name=f"A{it}")
            nc.scalar.activation(A[:], v["tps"][:], AF.Sign,
                                 bias=v["mx"][:], scale=-1.0)
            bn = work.tile([128, D + 1], FP16, tag="bn", bufs=3, name=f"bn{it}")
            nc.scalar.activation(bn[:, 0:D], v["b16"][:], AF.Copy,
                                 scale=grp[g]["s4"][:, j:j + 1])
            nc.gpsimd.memset(bn[:, D:D + 1], 1.0)
            v["A"], v["bn"] = A, bn

        def stage_segsum(it):
            v = st.pop(it)
            if (it & 3) == 3:
                grp.pop(it >> 2, None)
            A, bn = v["A"], v["bn"]
            for kt in range(4):
                nc.tensor.matmul(
                    acc[kt][:], lhsT=A[:, kt * 128:(kt + 1) * 128], rhs=bn[:],
                    start=(it == 0), stop=(it == NT - 1),
                )

        SKEW = 9
        for i in range(NT + SKEW):
            if i < NT:
                stage_load(i)
            if 0 <= i - 1 < NT:
                stage_stat(i - 1)
                if (i - 1) & 3 == 3:
                    stage_stat4((i - 1) >> 2)
            if 0 <= i - 2 < NT and (i - 2) & 3 == 3:
                stage_stat4b((i - 2) >> 2)
            if 0 <= i - 6 < NT:
                stage_scores(i - 6)
            if 0 <= i - 7 < NT:
                stage_select(i - 7)
            if 0 <= i - 8 < NT:
                stage_onehot(i - 8)
            if 0 <= i - 9 < NT:
                stage_segsum(i - 9)

        # ---------------- drain accumulators ----------------
        for kt in range(4):
            osb = work.tile([128, D + 1], F32, tag="osb", name=f"osb{kt}")
            nc.vector.tensor_copy(osb[:], acc[kt][:])
            nc.sync.dma_start(part_d[kt * 128:(kt + 1) * 128, :], osb[:])


def build_nc(debug=False):
    nc = bacc.Bacc("TRN2", target_bir_lowering=False, debug=debug,
                   num_devices=NCORES)
    bT_d = nc.dram_tensor("bT", [D, TPC], F32R, kind="ExternalInput").ap()
    btk_d = nc.dram_tensor("btk", [TPC, D], FP16, kind="ExternalInput").ap()
    pT_d = nc.dram_tensor("pT", [D, K], F32R, kind="ExternalInput").ap()
    q6_d = nc.dram_tensor("q6", [24, K], F32R, kind="ExternalInput").ap()
    id_d = nc.dram_tensor("idn", [128, 128], F32R, kind="ExternalInput").ap()
    c16_d = nc.dram_tensor("c16", [128, 2], F32R, kind="ExternalInput").ap()
    part_d = nc.dram_tensor("partial", [K, D + 1], F32, kind="ExternalOutput").ap()
    with tile.TileContext(nc) as tc:
        _body(tc, part_d, bT_d, btk_d, pT_d, q6_d, id_d, c16_d)
    nc.compile()
    return nc


_NC_CACHE = {}


def _get_nc():
    if "nc" not in _NC_CACHE:
        _NC_CACHE["nc"] = build_nc()
    return _NC_CACHE["nc"]


def _r12(a):
    """Round fp32 to 12 mantissa bits (nearest) — matches hw fp32r."""
    m, e = np.frexp(np.asarray(a, np.float32))
    return np.ldexp(np.round(m * 4096.0) / np.float32(4096.0), e).astype(np.float32)


def make_in_maps(batch, protos):
    flat = np.ascontiguousarray(batch.reshape(-1, D).astype(np.float32))
    p64 = protos.astype(np.float64)
    lens = np.sqrt(np.clip((p64 * p64).sum(-1), 0.0, None))
    p = p64 / np.clip(lens, 1.0, None)[:, None]
    q = (0.5 * (p * p).sum(-1)).astype(np.float32)           # [K]
    qh = _r12(q)
    ql = _r12(q - qh)
    q6 = np.zeros((24, K), np.float32)
    for j in range(4):
        q6[6 * j + j] = -qh        # pairs with delta_j row of tcol^T
        q6[6 * j + 4] = -qh        # pairs with const 16
        q6[6 * j + 5] = -ql        # pairs with const 16
    pT = np.ascontiguousarray(_r12(p.astype(np.float32)).T)  # [D, K]
    ident = np.eye(128, dtype=np.float32)
    c16 = np.full((128, 2), 16.0, np.float32)
    maps = []
    for i in range(NCORES):
        chunk = flat[i * TPC:(i + 1) * TPC]
        maps.append({
            "bT": np.ascontiguousarray(_r12(chunk).T),       # [D, TPC] f32r
            "btk": chunk.astype(np.float16),                 # [TPC, D] fp16
            "pT": pT,
            "q6": q6,
            "idn": ident,
            "c16": c16,
        })
    return maps


def correct_partial(raw):
    """Device outputs raw[k] = sum over tokens NOT assigned to k of
    [bn_t | 1].  Each token appears in exactly K-1 rows, so
    total = sum_k(raw)/(K-1) and true sums = total - raw."""
    raw = np.asarray(raw, np.float64)
    tot = raw.sum(axis=0) / (K - 1)
    return tot[None, :] - raw


def finish(partials, protoSums, protoCounts):
    """Host-side reduce of per-core partials + K*D running-stat update."""
    total = np.zeros((K, D + 1), np.float64)
    for pr in partials:
        total += correct_partial(pr)
    batchSums = total[:, :D]
    counts = total[:, D]
    newSums = protoSums.astype(np.float64) + batchSums
    newCounts = protoCounts.astype(np.float64) + counts
    newProtos = newSums / np.clip(newCounts, 1.0, None)[:, None]
    lens = np.sqrt(np.clip((newProtos * newProtos).sum(-1), 0.0, None))
    newProtos = newProtos / np.clip(lens, 1.0, None)[:, None]
    return newProtos.astype(np.float32)


def kernel(batch, protos, protoSums, protoCounts):
    nc = _get_nc()
    in_maps = make_in_maps(np.asarray(batch), np.asarray(protos))
    res = run_bass_kernel_spmd(nc, in_maps, list(range(NCORES)))
    partials = [r["partial"] for r in res.results]
    return finish(partials, np.asarray(protoSums), np.asarray(protoCounts))


if __name__ == "__main__":
    nc = build_nc()
    print("built + compiled OK")


# revision 7
# speedup vs baseline: 1.0189x; 1.0189x over previous
"""Trainium2 Bass kernel: CentroidModule (VQ codebook update).

Strategy (data-parallel over B across 8 NeuronCores, 8192 tokens/core):
  - Host prep (layout/dtype only): batch pre-transposed to d-major fp32r
    (12-bit-rounded fp32; runs on the PE at bf16 speed) for the scores
    lhsT, plus a token-major fp16 copy; protos normalized + transposed
    once (K*D work).
  - Scores on raw (unnormalized) tokens: argmin_k ||b/m - p_k||^2 ==
    argmax_k (S_tk - m_t*q_k) with S = batch @ p^T, q = 0.5||p||^2,
    m = max(1, len).  Per 128-token tile the PE does 3 fp32r matmuls
    into one PSUM tile [128, 512]: two 128-d blocks of S plus a C=6
    bias matmul whose lhsT holds transposed (m-16) columns and 16.0
    consts, rhs holds -q split hi/lo (12-bit pieces) -- bias exact to
    ~1e-6 while every PE pass runs at 1 cycle/row.
  - mx = row max (DVE reduce from PSUM); A = Sign(mx - u) on ACT =
    {0 at argmax, 1 else} (inverted one-hot, resolved on host);
    bn = fp16(b16 * s) via ACT Copy with per-partition scale.
  - Per-token stats: ss via one DVE scalar_tensor_tensor (b*1)*b with
    sum-accum; sqrt/recip/clip batched 4 tiles at a time to amortize
    engine fixed overheads.
  - segment sums: acc[K,257] += A^T @ [bn | 1] on PE, PSUM-accumulated
    over all 64 tiles (4 K-blocks, fp16).
  - Host: raw -> true sums via total/511 - raw (each token appears in
    exactly 511 of the inverted rows), add running stats, normalize.
"""

import numpy as np
from contextlib import ExitStack

import concourse.bacc as bacc
import concourse.mybir as mybir
import concourse.tile as tile
from concourse.bass_utils import run_bass_kernel_spmd

B, T, D, K = 64, 1024, 256, 512
NCORES = 8
TPC = (B * T) // NCORES      # tokens per core = 8192
NT = TPC // 128              # 64 token tiles per core
F32 = mybir.dt.float32
F32R = mybir.dt.float32r
FP16 = mybir.dt.float16
AF = mybir.ActivationFunctionType
OP = mybir.AluOpType


def _body(tc, part_d, bT_d, btk_d, pT_d, q6_d, id_d, c16_d):
    nc = tc.nc
    with ExitStack() as ctx:
        const = ctx.enter_context(tc.tile_pool(name="const", bufs=1))
        work = ctx.enter_context(tc.tile_pool(name="work", bufs=4))
        small = ctx.enter_context(tc.tile_pool(name="small", bufs=6))
        ppt = ctx.enter_context(tc.tile_pool(name="ppt", bufs=3, space="PSUM"))
        ppb = ctx.enter_context(tc.tile_pool(name="ppb", bufs=1, space="PSUM"))
        psums = ctx.enter_context(tc.tile_pool(name="psums", bufs=1, space="PSUM"))

        # ---------------- constants ----------------
        pT = [const.tile([128, K], F32R, tag=f"pT{h}", name=f"pT{h}")
              for h in (0, 1)]
        for h in (0, 1):
            nc.sync.dma_start(pT[h][:], pT_d[h * 128:(h + 1) * 128, :])
        q6 = [const.tile([6, K], F32R, tag=f"q6_{j}", name=f"q6_{j}")
              for j in range(4)]
        for j in range(4):
            nc.sync.dma_start(q6[j][:], q6_d[6 * j:6 * j + 6, :])
        ident = const.tile([128, 128], F32R, tag="ident", name="ident")
        nc.sync.dma_start(ident[:], id_d[:, :])

        # ---------------- segment-sum accumulators ----------------
        acc = [
            psums.tile([128, D + 1], F32, tag=f"acc{kt}", name=f"acc{kt}")
            for kt in range(4)
        ]

        st = {}
        grp = {}

        def stage_load(it):
            v = st.setdefault(it, {})
            bt0 = work.tile([128, 128], F32R, tag="bt0", bufs=10, name=f"bt0_{it}")
            bt1 = work.tile([128, 128], F32R, tag="bt1", bufs=10, name=f"bt1_{it}")
            nc.sync.dma_start(bt0[:], bT_d[0:128, it * 128:(it + 1) * 128])
            nc.sync.dma_start(bt1[:], bT_d[128:256, it * 128:(it + 1) * 128])
            b16 = work.tile([128, D], FP16, tag="b16", bufs=13, name=f"b16_{it}")
            nc.sync.dma_start(b16[:], btk_d[it * 128:(it + 1) * 128, :])
            v["bt0"], v["bt1"], v["b16"] = bt0, bt1, b16

        def stage_stat(it):
            g, j = it >> 2, it & 3
            if j == 0:
                gv = grp.setdefault(g, {})
                gv["ss4"] = small.tile([128, 4], F32, tag="ss4", bufs=3,
                                       name=f"ss4_{g}")
                gv["tcol"] = work.tile([128, 6], F32R, tag="tcol", bufs=2,
                                       name=f"tcol_{g}")
                nc.sync.dma_start(gv["tcol"][:, 4:6], c16_d[:, :])
            gv = grp[g]
            sq = work.tile([128, D], FP16, tag="sq", bufs=2, name=f"sq{it}")
            nc.vector.scalar_tensor_tensor(
                sq[:], st[it]["b16"][:], 1.0, st[it]["b16"][:],
                op0=OP.mult, op1=OP.mult, accum_out=gv["ss4"][:, j:j + 1],
            )

        def stage_stat4(g):
            gv = grp[g]
            ssc4 = small.tile([128, 4], F32, tag="ssc4", bufs=3, name=f"ssc4_{g}")
            nc.gpsimd.tensor_scalar_max(ssc4[:], gv["ss4"][:], 1.0)
            m4 = small.tile([128, 4], F32, tag="m4", bufs=3, name=f"m4_{g}")
            nc.scalar.activation(m4[:], ssc4[:], AF.Sqrt)
            s4 = small.tile([128, 4], F32, tag="s4", bufs=3, name=f"s4_{g}")
            nc.vector.reciprocal(s4[:], m4[:])
            nc.gpsimd.tensor_scalar_add(gv["tcol"][:, 0:4], m4[:], -16.0)
            gv["m4"], gv["s4"] = m4, s4

        def stage_stat4b(g):
            gv = grp[g]
            tct = ppb.tile([6, 128], F32R, tag="tct", name=f"tct_{g}")
            nc.tensor.transpose(tct[:], gv["tcol"][:], ident[:])
            tctr = work.tile([6, 128], F32R, tag="tctr", bufs=2, name=f"tctr_{g}")
            nc.vector.tensor_copy(tctr[:], tct[:])
            gv["tctr"] = tctr

        def stage_scores(it):
            v = st[it]
            g, j = it >> 2, it & 3
            tps = ppt.tile([128, K], F32, tag="tps", name=f"tps{it}")
            nc.tensor.matmul(tps[:], lhsT=v["bt0"][:], rhs=pT[0][:],
                             start=True, stop=False)
            nc.tensor.matmul(tps[:], lhsT=v["bt1"][:], rhs=pT[1][:],
                             start=False, stop=False)
            nc.tensor.matmul(tps[:], lhsT=grp[g]["tctr"][:], rhs=q6[j][:],
                             start=False, stop=True)
            v["tps"] = tps

        def stage_select(it):
            v = st[it]
            mx = small.tile([128, 1], F32, tag="mx", name=f"mx{it}")
            nc.vector.reduce_max(mx[:], v["tps"][:], axis=mybir.AxisListType.X)
            v["mx"] = mx

        def stage_onehot(it):
            v = st[it]
            g, j = it >> 2, it & 3
            A = work.tile([128, K], FP16, tag="A", bufs=3, name=f"A{it}")
            nc.scalar.activation(A[:], v["tps"][:], AF.Sign,
                                 bias=v["mx"][:], scale=-1.0)
            bn = work.tile([128, D + 1], FP16, tag="bn", bufs=3, name=f"bn{it}")
            nc.gpsimd.tensor_tensor(
                bn[:, 0:D], v["b16"][:],
                grp[g]["s4"][:, j:j + 1].broadcast_to([128, D]), OP.mult)
            nc.gpsimd.memset(bn[:, D:D + 1], 1.0)
            v["A"], v["bn"] = A, bn

        def stage_segsum(it):
            v = st.pop(it)
            if (it & 3) == 3:
                grp.pop(it >> 2, None)
            A, bn = v["A"], v["bn"]
            for kt in range(4):
                nc.tensor.matmul(
                    acc[kt][:], lhsT=A[:, kt * 128:(kt + 1) * 128], rhs=bn[:],
                    start=(it == 0), stop=(it == NT - 1),
                )

        SKEW = 11
        for i in range(NT + SKEW):
            if i < NT:
                stage_load(i)
            if 0 <= i - 1 < NT:
                stage_stat(i - 1)
                if (i - 1) & 3 == 3:
                    stage_stat4((i - 1) >> 2)
            if 0 <= i - 3 < NT and (i - 3) & 3 == 3:
                stage_stat4b((i - 3) >> 2)
            if 0 <= i - 8 < NT:
                stage_scores(i - 8)
            if 0 <= i - 9 < NT:
                stage_select(i - 9)
            if 0 <= i - 10 < NT:
                stage_onehot(i - 10)
            if 0 <= i - 11 < NT:
                stage_segsum(i - 11)

        # ---------------- drain accumulators ----------------
        for kt in range(4):
            osb = work.tile([128, D + 1], F32, tag="osb", name=f"osb{kt}")
            nc.vector.tensor_copy(osb[:], acc[kt][:])
            nc.sync.dma_start(part_d[kt * 128:(kt + 1) * 128, :], osb[:])


def build_nc(debug=False):
    nc = bacc.Bacc("TRN2", target_bir_lowering=False, debug=debug,
                   num_devices=NCORES)
    bT_d = nc.dram_tensor("bT", [D, TPC], F32R, kind="ExternalInput").ap()
    btk_d = nc.dram_tensor("btk", [TPC, D], FP16, kind="ExternalInput").ap()
    pT_d = nc.dram_tensor("pT", [D, K], F32R, kind="ExternalInput").ap()
    q6_d = nc.dram_tensor("q6", [24, K], F32R, kind="ExternalInput").ap()
    id_d = nc.dram_tensor("idn", [128, 128], F32R, kind="ExternalInput").ap()
    c16_d = nc.dram_tensor("c16", [128, 2], F32R, kind="ExternalInput").ap()
    part_d = nc.dram_tensor("partial", [K, D + 1], F32, kind="ExternalOutput").ap()
    with tile.TileContext(nc) as tc:
        _body(tc, part_d, bT_d, btk_d, pT_d, q6_d, id_d, c16_d)
    nc.compile()
    return nc


_NC_CACHE = {}


def _get_nc():
    if "nc" not in _NC_CACHE:
        _NC_CACHE["nc"] = build_nc()
    return _NC_CACHE["nc"]


def _r12(a):
    """Round fp32 to 12 mantissa bits (nearest) — matches hw fp32r."""
    m, e = np.frexp(np.asarray(a, np.float32))
    return np.ldexp(np.round(m * 4096.0) / np.float32(4096.0), e).astype(np.float32)


def make_in_maps(batch, protos):
    flat = np.ascontiguousarray(batch.reshape(-1, D).astype(np.float32))
    p64 = protos.astype(np.float64)
    lens = np.sqrt(np.clip((p64 * p64).sum(-1), 0.0, None))
    p = p64 / np.clip(lens, 1.0, None)[:, None]
    q = (0.5 * (p * p).sum(-1)).astype(np.float32)           # [K]
    qh = _r12(q)
    ql = _r12(q - qh)
    q6 = np.zeros((24, K), np.float32)
    for j in range(4):
        q6[6 * j + j] = -qh        # pairs with delta_j row of tcol^T
        q6[6 * j + 4] = -qh        # pairs with const 16
        q6[6 * j + 5] = -ql        # pairs with const 16
    pT = np.ascontiguousarray(_r12(p.astype(np.float32)).T)  # [D, K]
    ident = np.eye(128, dtype=np.float32)
    c16 = np.full((128, 2), 16.0, np.float32)
    maps = []
    for i in range(NCORES):
        chunk = flat[i * TPC:(i + 1) * TPC]
        maps.append({
            "bT": np.ascontiguousarray(_r12(chunk).T),       # [D, TPC] f32r
            "btk": chunk.astype(np.float16),                 # [TPC, D] fp16
            "pT": pT,
            "q6": q6,
            "idn": ident,
            "c16": c16,
        })
    return maps


def correct_partial(raw):
    """Device outputs raw[k] = sum over tokens NOT assigned to k of
    [bn_t | 1].  Each token appears in exactly K-1 rows, so
    total = sum_k(raw)/(K-1) and true sums = total - raw."""
    raw = np.asarray(raw, np.float64)
    tot = raw.sum(axis=0) / (K - 1)
    return tot[None, :] - raw


def finish(partials, protoSums, protoCounts):
    """Host-side reduce of per-core partials + K*D running-stat update."""
    total = np.zeros((K, D + 1), np.float64)
    for pr in partials:
        total += correct_partial(pr)
    batchSums = total[:, :D]
    counts = total[:, D]
    newSums = protoSums.astype(np.float64) + batchSums
    newCounts = protoCounts.astype(np.float64) + counts
    newProtos = newSums / np.clip(newCounts, 1.0, None)[:, None]
    lens = np.sqrt(np.clip((newProtos * newProtos).sum(-1), 0.0, None))
    newProtos = newProtos / np.clip(lens, 1.0, None)[:, None]
    return newProtos.astype(np.float32)


def kernel(batch, protos, protoSums, protoCounts):
    nc = _get_nc()
    in_maps = make_in_maps(np.asarray(batch), np.asarray(protos))
    res = run_bass_kernel_spmd(nc, in_maps, list(range(NCORES)))
    partials = [r["partial"] for r in res.results]
    return finish(partials, np.asarray(protoSums), np.asarray(protoCounts))


if __name__ == "__main__":
    nc = build_nc()
    print("built + compiled OK")


# revision 8
# speedup vs baseline: 1.0232x; 1.0042x over previous
"""Trainium2 Bass kernel: CentroidModule (VQ codebook update).

Strategy (data-parallel over B across 8 NeuronCores, 8192 tokens/core):
  - Host prep (layout/dtype only): batch pre-transposed to d-major fp32r
    (12-bit-rounded fp32; runs on the PE at bf16 speed) for the scores
    lhsT, plus a token-major fp16 copy; protos normalized + transposed
    once (K*D work).
  - Scores on raw (unnormalized) tokens: argmin_k ||b/m - p_k||^2 ==
    argmax_k (S_tk - m_t*q_k) with S = batch @ p^T, q = 0.5||p||^2,
    m = max(1, len).  Per 128-token tile the PE does 3 fp32r matmuls
    into one PSUM tile [128, 512]: two 128-d blocks of S plus a C=6
    bias matmul whose lhsT holds transposed (m-16) columns and 16.0
    consts, rhs holds -q split hi/lo (12-bit pieces) -- bias exact to
    ~1e-6 while every PE pass runs at 1 cycle/row.
  - mx = row max (DVE reduce from PSUM); A = Sign(mx - u) on ACT =
    {0 at argmax, 1 else} (inverted one-hot, resolved on host);
    bn = fp16(b16 * s) via ACT Copy with per-partition scale.
  - Per-token stats: ss via one DVE scalar_tensor_tensor (b*1)*b with
    sum-accum; sqrt/recip/clip batched 4 tiles at a time to amortize
    engine fixed overheads.
  - segment sums: acc[K,257] += A^T @ [bn | 1] on PE, PSUM-accumulated
    over all 64 tiles (4 K-blocks, fp16).
  - Host: raw -> true sums via total/511 - raw (each token appears in
    exactly 511 of the inverted rows), add running stats, normalize.
"""

import numpy as np
from contextlib import ExitStack

import concourse.bacc as bacc
import concourse.mybir as mybir
import concourse.tile as tile
from concourse.bass_utils import run_bass_kernel_spmd

B, T, D, K = 64, 1024, 256, 512
NCORES = 8
TPC = (B * T) // NCORES      # tokens per core = 8192
NT = TPC // 128              # 64 token tiles per core
F32 = mybir.dt.float32
F32R = mybir.dt.float32r
FP16 = mybir.dt.float16
AF = mybir.ActivationFunctionType
OP = mybir.AluOpType


def _body(tc, part_d, bT_d, btk_d, pT_d, q6_d, id_d, c16_d):
    nc = tc.nc
    with ExitStack() as ctx:
        const = ctx.enter_context(tc.tile_pool(name="const", bufs=1))
        work = ctx.enter_context(tc.tile_pool(name="work", bufs=4))
        small = ctx.enter_context(tc.tile_pool(name="small", bufs=6))
        ppt = ctx.enter_context(tc.tile_pool(name="ppt", bufs=3, space="PSUM"))
        ppb = ctx.enter_context(tc.tile_pool(name="ppb", bufs=1, space="PSUM"))
        psums = ctx.enter_context(tc.tile_pool(name="psums", bufs=1, space="PSUM"))

        # ---------------- constants ----------------
        pT = [const.tile([128, K], F32R, tag=f"pT{h}", name=f"pT{h}")
              for h in (0, 1)]
        for h in (0, 1):
            nc.sync.dma_start(pT[h][:], pT_d[h * 128:(h + 1) * 128, :])
        q6 = [const.tile([6, K], F32R, tag=f"q6_{j}", name=f"q6_{j}")
              for j in range(4)]
        for j in range(4):
            nc.sync.dma_start(q6[j][:], q6_d[6 * j:6 * j + 6, :])
        ident = const.tile([128, 128], F32R, tag="ident", name="ident")
        nc.sync.dma_start(ident[:], id_d[:, :])

        # ---------------- segment-sum accumulators ----------------
        acc = [
            psums.tile([128, D + 1], F32, tag=f"acc{kt}", name=f"acc{kt}")
            for kt in range(4)
        ]

        st = {}
        grp = {}

        def stage_load(it):
            v = st.setdefault(it, {})
            bt0 = work.tile([128, 128], F32R, tag="bt0", bufs=14, name=f"bt0_{it}")
            bt1 = work.tile([128, 128], F32R, tag="bt1", bufs=14, name=f"bt1_{it}")
            nc.sync.dma_start(bt0[:], bT_d[0:128, it * 128:(it + 1) * 128])
            nc.sync.dma_start(bt1[:], bT_d[128:256, it * 128:(it + 1) * 128])
            b16 = work.tile([128, D], FP16, tag="b16", bufs=16, name=f"b16_{it}")
            nc.sync.dma_start(b16[:], btk_d[it * 128:(it + 1) * 128, :])
            v["bt0"], v["bt1"], v["b16"] = bt0, bt1, b16

        def stage_stat(it):
            g, j = it >> 2, it & 3
            if j == 0:
                gv = grp.setdefault(g, {})
                gv["ss4"] = small.tile([128, 4], F32, tag="ss4", bufs=4,
                                       name=f"ss4_{g}")
                gv["tcol"] = work.tile([128, 6], F32R, tag="tcol", bufs=3,
                                       name=f"tcol_{g}")
                nc.sync.dma_start(gv["tcol"][:, 4:6], c16_d[:, :])
            gv = grp[g]
            sq = work.tile([128, D], FP16, tag="sq", bufs=2, name=f"sq{it}")
            nc.vector.scalar_tensor_tensor(
                sq[:], st[it]["b16"][:], 1.0, st[it]["b16"][:],
                op0=OP.mult, op1=OP.mult, accum_out=gv["ss4"][:, j:j + 1],
            )

        def stage_stat4(g):
            gv = grp[g]
            ssc4 = small.tile([128, 4], F32, tag="ssc4", bufs=4, name=f"ssc4_{g}")
            nc.gpsimd.tensor_scalar_max(ssc4[:], gv["ss4"][:], 1.0)
            m4 = small.tile([128, 4], F32, tag="m4", bufs=4, name=f"m4_{g}")
            nc.scalar.activation(m4[:], ssc4[:], AF.Sqrt)
            s4 = small.tile([128, 4], F32, tag="s4", bufs=4, name=f"s4_{g}")
            nc.vector.reciprocal(s4[:], m4[:])
            nc.gpsimd.tensor_scalar_add(gv["tcol"][:, 0:4], m4[:], -16.0)
            gv["m4"], gv["s4"] = m4, s4

        def stage_stat4b(g):
            gv = grp[g]
            tct = ppb.tile([6, 128], F32R, tag="tct", name=f"tct_{g}")
            nc.tensor.transpose(tct[:], gv["tcol"][:], ident[:])
            tctr = work.tile([6, 128], F32R, tag="tctr", bufs=3, name=f"tctr_{g}")
            nc.vector.tensor_copy(tctr[:], tct[:])
            gv["tctr"] = tctr

        def stage_scores(it):
            v = st[it]
            g, j = it >> 2, it & 3
            tps = ppt.tile([128, K], F32, tag="tps", name=f"tps{it}")
            nc.tensor.matmul(tps[:], lhsT=v["bt0"][:], rhs=pT[0][:],
                             start=True, stop=False)
            nc.tensor.matmul(tps[:], lhsT=v["bt1"][:], rhs=pT[1][:],
                             start=False, stop=False)
            nc.tensor.matmul(tps[:], lhsT=grp[g]["tctr"][:], rhs=q6[j][:],
                             start=False, stop=True)
            v["tps"] = tps

        def stage_select(it):
            v = st[it]
            mx = small.tile([128, 1], F32, tag="mx", name=f"mx{it}")
            nc.vector.reduce_max(mx[:], v["tps"][:], axis=mybir.AxisListType.X)
            v["mx"] = mx

        def stage_onehot(it):
            v = st[it]
            g, j = it >> 2, it & 3
            A = work.tile([128, K], FP16, tag="A", bufs=3, name=f"A{it}")
            nc.scalar.activation(A[:], v["tps"][:], AF.Sign,
                                 bias=v["mx"][:], scale=-1.0)
            bn = work.tile([128, D + 1], FP16, tag="bn", bufs=3, name=f"bn{it}")
            nc.gpsimd.tensor_tensor(
                bn[:, 0:D], v["b16"][:],
                grp[g]["s4"][:, j:j + 1].broadcast_to([128, D]), OP.mult)
            nc.gpsimd.memset(bn[:, D:D + 1], 1.0)
            v["A"], v["bn"] = A, bn

        def stage_segsum(it):
            v = st.pop(it)
            if (it & 3) == 3:
                grp.pop(it >> 2, None)
            A, bn = v["A"], v["bn"]
            for kt in range(4):
                nc.tensor.matmul(
                    acc[kt][:], lhsT=A[:, kt * 128:(kt + 1) * 128], rhs=bn[:],
                    start=(it == 0), stop=(it == NT - 1),
                )

        SKEW = 15
        for i in range(NT + SKEW):
            if i < NT:
                stage_load(i)
            if 0 <= i - 1 < NT:
                stage_stat(i - 1)
                if (i - 1) & 3 == 3:
                    stage_stat4((i - 1) >> 2)
            if 0 <= i - 4 < NT and (i - 4) & 3 == 3:
                stage_stat4b((i - 4) >> 2)
            if 0 <= i - 12 < NT:
                stage_scores(i - 12)
            if 0 <= i - 13 < NT:
                stage_select(i - 13)
            if 0 <= i - 14 < NT:
                stage_onehot(i - 14)
            if 0 <= i - 15 < NT:
                stage_segsum(i - 15)

        # ---------------- drain accumulators ----------------
        for kt in range(4):
            osb = work.tile([128, D + 1], F32, tag="osb", name=f"osb{kt}")
            nc.vector.tensor_copy(osb[:], acc[kt][:])
            nc.sync.dma_start(part_d[kt * 128:(kt + 1) * 128, :], osb[:])


def build_nc(debug=False):
    nc = bacc.Bacc("TRN2", target_bir_lowering=False, debug=debug,
                   num_devices=NCORES)
    bT_d = nc.dram_tensor("bT", [D, TPC], F32R, kind="ExternalInput").ap()
    btk_d = nc.dram_tensor("btk", [TPC, D], FP16, kind="ExternalInput").ap()
    pT_d = nc.dram_tensor("pT", [D, K], F32R, kind="ExternalInput").ap()
    q6_d = nc.dram_tensor("q6", [24, K], F32R, kind="ExternalInput").ap()
    id_d = nc.dram_tensor("idn", [128, 128], F32R, kind="ExternalInput").ap()
    c16_d = nc.dram_tensor("c16", [128, 2], F32R, kind="ExternalInput").ap()
    part_d = nc.dram_tensor("partial", [K, D + 1], F32, kind="ExternalOutput").ap()
    with tile.TileContext(nc) as tc:
        _body(tc, part_d, bT_d, btk_d, pT_d, q6_d, id_d, c16_d)
    nc.compile()
    return nc


_NC_CACHE = {}


def _get_nc():
    if "nc" not in _NC_CACHE:
        _NC_CACHE["nc"] = build_nc()
    return _NC_CACHE["nc"]


def _r12(a):
    """Round fp32 to 12 mantissa bits (nearest) — matches hw fp32r."""
    m, e = np.frexp(np.asarray(a, np.float32))
    return np.ldexp(np.round(m * 4096.0) / np.float32(4096.0), e).astype(np.float32)


def make_in_maps(batch, protos):
    flat = np.ascontiguousarray(batch.reshape(-1, D).astype(np.float32))
    p64 = protos.astype(np.float64)
    lens = np.sqrt(np.clip((p64 * p64).sum(-1), 0.0, None))
    p = p64 / np.clip(lens, 1.0, None)[:, None]
    q = (0.5 * (p * p).sum(-1)).astype(np.float32)           # [K]
    qh = _r12(q)
    ql = _r12(q - qh)
    q6 = np.zeros((24, K), np.float32)
    for j in range(4):
        q6[6 * j + j] = -qh        # pairs with delta_j row of tcol^T
        q6[6 * j + 4] = -qh        # pairs with const 16
        q6[6 * j + 5] = -ql        # pairs with const 16
    pT = np.ascontiguousarray(_r12(p.astype(np.float32)).T)  # [D, K]
    ident = np.eye(128, dtype=np.float32)
    c16 = np.full((128, 2), 16.0, np.float32)
    maps = []
    for i in range(NCORES):
        chunk = flat[i * TPC:(i + 1) * TPC]
        maps.append({
            "bT": np.ascontiguousarray(_r12(chunk).T),       # [D, TPC] f32r
            "btk": chunk.astype(np.float16),                 # [TPC, D] fp16
            "pT": pT,
            "q6": q6,
            "idn": ident,
            "c16": c16,
        })
    return maps


def correct_partial(raw):
    """Device outputs raw[k] = sum over tokens NOT assigned to k of
    [bn_t | 1].  Each token appears in exactly K-1 rows, so
    total = sum_k(raw)/(K-1) and true sums = total - raw."""
    raw = np.asarray(raw, np.float64)
    tot = raw.sum(axis=0) / (K - 1)
    return tot[None, :] - raw


def finish(partials, protoSums, protoCounts):
    """Host-side reduce of per-core partials + K*D running-stat update."""
    total = np.zeros((K, D + 1), np.float64)
    for pr in partials:
        total += correct_partial(pr)
    batchSums = total[:, :D]
    counts = total[:, D]
    newSums = protoSums.astype(np.float64) + batchSums
    newCounts = protoCounts.astype(np.float64) + counts
    newProtos = newSums / np.clip(newCounts, 1.0, None)[:, None]
    lens = np.sqrt(np.clip((newProtos * newProtos).sum(-1), 0.0, None))
    newProtos = newProtos / np.clip(lens, 1.0, None)[:, None]
    return newProtos.astype(np.float32)


def kernel(batch, protos, protoSums, protoCounts):
    nc = _get_nc()
    in_maps = make_in_maps(np.asarray(batch), np.asarray(protos))
    res = run_bass_kernel_spmd(nc, in_maps, list(range(NCORES)))
    partials = [r["partial"] for r in res.results]
    return finish(partials, np.asarray(protoSums), np.asarray(protoCounts))


if __name__ == "__main__":
    nc = build_nc()
    print("built + compiled OK")
